# revision 1
# baseline (speedup 1.0000x reference)
"""Trainium2 Bass kernel for nn_CG_MSA_M (cross-gated multi-head channel attention).

Sharding: data-parallel over batch (8 samples -> 8 cores), weights replicated.

Per-core algorithm (one sample, C=96 channels, N=128x128 positions):
  - x,y are staged into zero-padded 130x130 "canvases" (bf16) so every 3x3
    shift is a pure access-pattern offset with exact zero padding.
  - The depthwise 3x3 convs are folded into the producing 1x1 convs on the
    TensorEngine: q = sum_tap (W_qv . diag(dw_tap)) @ x_shift(tap), i.e. 9
    accumulating matmuls with host-precomposed weights.  Same for k, v_, v0.
  - Channel-attention Gram matrices (contract over the 16384 positions) are
    accumulated chunk-wise via PE transposes + matmuls; L2 norms come from the
    diagonals of Gqq/Gkk.
  - v = 3x3 conv over [v0; v_] canvases: 18 accumulating matmuls per chunk.
  - Output = [W_proj@A | W_posX | W_posY] @ [v; x; y] fused into one matmul
    group per chunk, DMA'd straight from PSUM to DRAM.
"""

import numpy as np
import ml_dtypes

import concourse.bass as bass
import concourse.tile as tile
from concourse import bacc, mybir
from concourse.bass_utils import run_bass_kernel_spmd

BF16 = mybir.dt.float16  # fp16: same PE rate as bf16, 10-bit mantissa
F32 = mybir.dt.float32

B, C, H, W, HEADS = 8, 96, 128, 128, 6
CH = C // HEADS  # 16
N = H * W  # 16384
WC = W + 2  # canvas row stride 130
CANV = (H + 2) * WC  # 16900
NCHUNK = 512  # positions per chunk (4 rows)
RPC = NCHUNK // W  # rows per chunk = 4
NCB = N // NCHUNK  # 32 chunks

# module-level knob so test.py can request a profiled run
TRACE = False
LAST_RESULTS = None


def _bf16(a):
    return np.ascontiguousarray(a.astype(np.float16))


def _f32(a):
    return np.ascontiguousarray(a.astype(np.float32))


def _prep_weights(w_pos, w_qv, w_qv_dw, w_kv, w_kv_dw, w_proj, w_fuse, b_fuse,
                  temperature):
    """Host-side weight composition (numpy)."""
    w_pos = w_pos[:, :, 0, 0]      # [192,192]
    w_qv = w_qv[:, :, 0, 0]        # [192,96]
    w_kv = w_kv[:, :, 0, 0]        # [192,96]
    w_proj = w_proj[:, :, 0, 0]    # [192,96]
    dwq = w_qv_dw[:, 0].reshape(2 * C, 9)   # [192,9]
    dwk = w_kv_dw[:, 0].reshape(2 * C, 9)   # [192,9]

    # wx9[i, t, c] = w_qv[c, i] * dwq[c, t]   c in [0,192) (q-half then v_-half)
    wx9 = np.einsum('ci,ct->itc', w_qv, dwq)             # [96, 9, 192]
    wy9 = np.einsum('ci,ct->itc', w_kv, dwk)             # [96, 9, 192]

    # fuse 3x3: wf[j, half*9+t, o] = w_fuse[o, 96*half + j, t]
    wfr = w_fuse.reshape(C, 2 * C, 9)                    # [96 o, 192 j, 9 t]
    wf = np.empty((C, 18, C), np.float32)                # [j, s, o]
    for half in range(2):
        # [o, j, t] -> [j, t, o]
        wf[:, half * 9:(half + 1) * 9, :] = np.transpose(
            wfr[:, half * C:(half + 1) * C, :], (1, 2, 0))

    wprojT = w_proj.T                                     # [96 c', 192 o]
    wposxT = w_pos[:, :C].T                               # [96 i, 192 o]
    wposyT = w_pos[:, C:].T                               # [96 i, 192 o]

    temp_row = np.repeat(temperature.reshape(HEADS), CH).reshape(C, 1)
    bfuse = b_fuse.reshape(C, 1)

    return {
        "wx9": _bf16(wx9.reshape(C, 9 * 2 * C)),
        "wy9": _bf16(wy9.reshape(C, 9 * 2 * C)),
        "wf": _bf16(wf.reshape(C, 18 * C)),
        "wprojT": _bf16(wprojT),
        "wposxT": _bf16(wposxT),
        "wposyT": _bf16(wposyT),
        "temp_row": _f32(temp_row),
        "bfuse": _f32(bfuse),
        "identf": _f32(np.eye(C)),
        "mask": _f32(np.kron(np.eye(HEADS), np.ones((CH, CH))) * 30000.0
                     - 30000.0),
    }


def _canvas_view(canvas_ap, cb, dy, dx, rows=RPC):
    """[96, rows, 128] view of a canvas for chunk cb shifted by (dy,dx) in {0,1,2}."""
    r = canvas_ap.rearrange("p (r c) -> p r c", c=WC)
    r0 = cb * RPC + dy
    return r[:, r0:r0 + rows, dx:dx + W]


def _dummy_out(tc, nc, out_d):
    with tc.tile_pool(name="dummy", bufs=2) as dp:
        for mt, (o0, osz) in enumerate(((0, 128), (128, 64))):
            for cb in range(NCB):
                t = dp.tile([osz, NCHUNK], F32, tag=f"d{mt}")
                nc.vector.memset(t[:], 0.0)
                nc.sync.dma_start(
                    out=out_d[o0:o0 + osz, cb * NCHUNK:(cb + 1) * NCHUNK],
                    in_=t[:])


def _build_nc():
    nc = bacc.Bacc(None, name="cg_msa")

    x_d = nc.dram_tensor("x", [C, N], F32, kind="ExternalInput")
    y_d = nc.dram_tensor("y", [C, N], F32, kind="ExternalInput")
    w_d = {}
    wspec = [("wx9", [C, 9 * 2 * C], BF16), ("wy9", [C, 9 * 2 * C], BF16),
             ("wf", [C, 18 * C], BF16), ("wprojT", [C, 2 * C], BF16),
             ("wposxT", [C, 2 * C], BF16), ("wposyT", [C, 2 * C], BF16),
             ("temp_row", [C, 1], F32), ("bfuse", [C, 1], F32),
             ("identf", [C, C], F32),
             ("mask", [C, C], F32)]
    for nm, shp, dt in wspec:
        w_d[nm] = nc.dram_tensor(nm, shp, dt, kind="ExternalInput")
    out_d = nc.dram_tensor("out", [2 * C, N], F32, kind="ExternalOutput")

    with tile.TileContext(nc) as tc:
        _emit(tc, nc, x_d, y_d, w_d, out_d)
    nc.finalize()
    return nc


def _emit(tc, nc, x_d, y_d, w_d, out_d):
    import os
    from contextlib import ExitStack
    PH = os.environ.get("K_PHASES", "BCDEF")
    ctx = ExitStack()
    with ctx:
        const = ctx.enter_context(tc.tile_pool(name="const", bufs=1))
        canv = ctx.enter_context(tc.tile_pool(name="canv", bufs=1))
        big = ctx.enter_context(tc.tile_pool(name="big", bufs=1))

        # ---- weights to SBUF ----
        wsb = {}
        for wi, nm in enumerate(
                ("wx9", "wy9", "wf", "wprojT", "wposxT", "wposyT",
                 "temp_row", "bfuse", "identf", "mask")):
            t = const.tile(list(w_d[nm].shape), w_d[nm].dtype, tag=f"w_{nm}")
            (nc.sync if wi % 2 == 0 else nc.scalar).dma_start(
                out=t[:], in_=w_d[nm][:])
            wsb[nm] = t

        # ---- canvases ----
        xc = canv.tile([C, CANV], BF16)
        yc = canv.tile([C, CANV], BF16)
        vc0 = canv.tile([C, CANV], BF16)
        vc_ = canv.tile([C, CANV], BF16)
        for t in (xc, yc, vc0, vc_):
            # zero only the borders; the interior is fully overwritten.
            nc.gpsimd.memset(t[:, 0:WC], 0.0)                    # top pad row
            nc.gpsimd.memset(t[:, (H + 1) * WC:CANV], 0.0)       # bottom pad row
            # right pad col of row r and left pad col of row r+1 are adjacent:
            # cols {129,130}, {259,260}, ... one strided 2-wide memset.
            side = t[:].rearrange("p (r c) -> p r c", c=WC)
            nc.gpsimd.memset(side[:, 0:H + 1, W + 1:W + 2], 0.0)
            nc.gpsimd.memset(side[:, 1:H + 2, 0:1], 0.0)

        v_sb = big.tile([C, N], BF16)
        stats = ctx.enter_context(tc.tile_pool(name="stats", bufs=1))
        nqp = stats.tile([C, NCB], F32, tag="nqp")
        nkp = stats.tile([C, NCB], F32, tag="nkp")

        # ---- phase B: stream x,y in, convert to bf16 canvases ----
        if "B" not in PH:
            _dummy_out(tc, nc, out_d)
            return
        from contextlib import ExitStack as _ES2
        ioctx = _ES2()
        io = ioctx.enter_context(tc.tile_pool(name="io", bufs=7))

        def load_chunk(cb):
            if cb >= NCB:
                return
            for si, (src_d, cv) in enumerate(((x_d, xc), (y_d, yc))):
                tin = io.tile([C, NCHUNK], F32, tag="in")
                eng = nc.sync if si == 0 else nc.scalar
                eng.dma_start(
                    out=tin[:], in_=src_d[:, cb * NCHUNK:(cb + 1) * NCHUNK])
                dst = _canvas_view(cv[:], cb, 1, 1)
                nc.vector.tensor_copy(
                    dst, tin[:].rearrange("p (r c) -> p r c", c=W))

        for cb in range(4):
            load_chunk(cb)

        # PE warmup during the startup bubble: ramps the clock gate before
        # real work arrives; reads a memset scratch tile, no input deps.
        with tc.tile_pool(name="warm", bufs=1) as warm, \
             tc.tile_pool(name="warmps", bufs=1, space="PSUM") as warmps:
            wsc = warm.tile([C, NCHUNK], BF16)
            nc.vector.memset(wsc[:], 0.0)
            wps = warmps.tile([C, NCHUNK], F32)
            for _ in range(16):
                nc.tensor.matmul(wps[:], wsc[:, 0:C], wsc[:],
                                 start=True, stop=True)

        # ---- phase C: q,k,v_,v0 + gram accumulation ----
        if "C" not in PH:
            _dummy_out(tc, nc, out_d)
            return
        from contextlib import ExitStack as _ES
        gctx = _ES()
        gpool = gctx.enter_context(tc.tile_pool(name="gps", bufs=1, space="PSUM"))
        g_ps = gpool.tile([C, C], F32)  # Gqk

        def fuse_chunk(psE, cb):
            ps = psE.tile([C, NCHUNK], F32, tag="v")
            k = 0
            for half, cv in ((0, vc0), (1, vc_)):
                for t in range(9):
                    dy, dx = t // 3, t % 3
                    s = half * 9 + t
                    nc.tensor.matmul(
                        ps[:], wsb["wf"][:, s * C:(s + 1) * C],
                        _canvas_view(cv[:], cb, dy, dx),
                        start=(k == 0), stop=(k == 17))
                    k += 1
            nc.scalar.activation(
                v_sb[:, cb * NCHUNK:(cb + 1) * NCHUNK], ps[:],
                mybir.ActivationFunctionType.Identity,
                bias=wsb["bfuse"][:], scale=1.0)

        with tc.tile_pool(name="psC", bufs=5, space="PSUM") as psC, \
             tc.tile_pool(name="psE", bufs=2, space="PSUM") as psE, \
             tc.tile_pool(name="stC", bufs=8) as stC, \
             tc.tile_pool(name="sqp", bufs=2) as sqp, \
             tc.tile_pool(name="stT", bufs=4) as stT:
            def gram_chunk(tps, cb):
                for j in range(RPC):
                    st = (cb == 0 and j == 0)
                    sp = (cb == NCB - 1 and j == RPC - 1)
                    nc.tensor.matmul(
                        g_ps[:], tps[:, j, 0, :], tps[:, j, 1, :],
                        start=st, stop=sp, skip_group_check=True)

            tps_prev = None
            for cb in range(NCB):
                load_chunk(cb + 4)
                outs_sb = {}
                for side, (cv, w9) in enumerate(
                        ((xc, wsb["wx9"]), (yc, wsb["wy9"]))):
                    for half in range(2):  # 0: q/k, 1: v_/v0
                        ps = psC.tile([C, NCHUNK], F32, tag="qv")
                        for t in range(9):
                            dy, dx = t // 3, t % 3
                            lhsT = w9[:, t * 2 * C + half * C:
                                      t * 2 * C + half * C + C]
                            nc.tensor.matmul(
                                ps[:], lhsT, _canvas_view(cv[:], cb, dy, dx),
                                start=(t == 0), stop=(t == 8))
                        if half == 0:
                            sb = stC.tile([C, NCHUNK], BF16, tag="qk")
                            nc.scalar.copy(out=sb[:], in_=ps[:])
                            sq = sqp.tile([C, NCHUNK], BF16, tag="sq")
                            npart = nqp if side == 0 else nkp
                            nc.scalar.activation(
                                sq[:], sb[:],
                                mybir.ActivationFunctionType.Square,
                                accum_out=npart[:, cb:cb + 1])
                            outs_sb[side] = sb
                        else:
                            cvv = vc_ if side == 0 else vc0
                            nc.vector.tensor_copy(
                                _canvas_view(cvv[:], cb, 1, 1),
                                ps[:].rearrange("p (r c) -> p r c", c=W))

                # xbar DMA transposes -> [128, j, {q,k}, 96]
                tps = stT.tile([W, RPC, 2, C], BF16)
                for s in range(2):
                    nc.scalar.dma_start_transpose(
                        tps[:, :, s, :], outs_sb[s][:])
                # gram + fuse trail one chunk so their deps are long ready
                if cb >= 1:
                    gram_chunk(tps_prev, cb - 1)
                    fuse_chunk(psE, cb - 1)
                tps_prev = tps
            gram_chunk(tps_prev, NCB - 1)
            fuse_chunk(psE, NCB - 1)
        ioctx.close()

        # ---- phase D: norms, softmax, M1T ----
        if "D" not in PH:
            gctx.close()
            _dummy_out(tc, nc, out_d)
            return
        smx = ctx.enter_context(tc.tile_pool(name="smx", bufs=1))
        with tc.tile_pool(name="psD", bufs=1, space="PSUM") as psD:
            g_sb = smx.tile([C, C], F32)
            nc.vector.tensor_copy(g_sb[:], g_ps[:])

            rr = {}
            for npart, nm in ((nqp, "q"), (nkp, "k")):
                nrm2 = smx.tile([C, 1], F32, tag=f"n{nm}")
                nc.vector.tensor_reduce(
                    nrm2[:], npart[:], axis=mybir.AxisListType.X,
                    op=mybir.AluOpType.add)
                nrm = smx.tile([C, 1], F32, tag=f"s{nm}")
                nc.scalar.sqrt(nrm[:], nrm2[:])
                nc.vector.tensor_scalar_max(nrm[:], nrm[:], 1e-12)
                rinv = smx.tile([C, 1], F32, tag=f"r{nm}")
                nc.vector.reciprocal(rinv[:], nrm[:])
                rr[nm] = rinv
            # fold temperature into rq
            nc.vector.tensor_tensor(
                rr["q"][:], rr["q"][:], wsb["temp_row"][:],
                mybir.AluOpType.mult)

            # rows [1,96] via PE transpose, then outer product R = rq (x) rk
            rows = {}
            for nm in ("q", "k"):
                rp = psD.tile([1, C], F32, tag="row")
                nc.tensor.transpose(rp[:], rr[nm][:], wsb["identf"][:])
                rs = smx.tile([1, C], F32, tag=f"row{nm}")
                nc.vector.tensor_copy(rs[:], rp[:])
                rows[nm] = rs
            r_ps = psD.tile([C, C], F32, tag="R")
            nc.tensor.matmul(r_ps[:], rows["q"][:], rows["k"][:])
            logits = smx.tile([C, C], F32)
            nc.vector.tensor_tensor(
                logits[:], g_sb[:], r_ps[:], mybir.AluOpType.mult)
            # additive block-diagonal mask: off-head-block = -30000 -> exp = 0
            nc.vector.tensor_tensor(
                logits[:], logits[:], wsb["mask"][:], mybir.AluOpType.add)

            mx = smx.tile([C, 1], F32)
            nc.vector.tensor_reduce(
                mx[:], logits[:], axis=mybir.AxisListType.X,
                op=mybir.AluOpType.max, negate=True)
            e = smx.tile([C, C], F32)
            nc.scalar.activation(
                e[:], logits[:], mybir.ActivationFunctionType.Exp,
                bias=mx[:], scale=1.0)
            s = smx.tile([C, 1], F32)
            nc.vector.tensor_reduce(
                s[:], e[:], axis=mybir.AxisListType.X, op=mybir.AluOpType.add)
            rs = smx.tile([C, 1], F32)
            nc.vector.reciprocal(rs[:], s[:])
            a_sb = smx.tile([C, C], BF16)
            nc.scalar.mul(a_sb[:], e[:], rs[:])

            m1_ps = psD.tile([C, 2 * C], F32, tag="m1")
            nc.tensor.matmul(m1_ps[:], a_sb[:], wsb["wprojT"][:])
            m1T = smx.tile([C, 2 * C], BF16)
            nc.vector.tensor_copy(m1T[:], m1_ps[:])
        gctx.close()

        # ---- phase F: out = M1 @ v + W_pos @ [x;y] ----
        if "F" not in PH:
            _dummy_out(tc, nc, out_d)
            return
        with tc.tile_pool(name="psF", bufs=4, space="PSUM") as psF, \
             tc.tile_pool(name="ostg", bufs=6) as ostg:
            for mt, (o0, osz) in enumerate(((0, 128), (128, 64))):
                for cb in range(NCB):
                    ps = psF.tile([osz, NCHUNK], F32, tag=f"o{mt}")
                    nc.tensor.matmul(
                        ps[:], wsb["wposxT"][:, o0:o0 + osz],
                        _canvas_view(xc[:], cb, 1, 1),
                        start=True, stop=False)
                    nc.tensor.matmul(
                        ps[:], wsb["wposyT"][:, o0:o0 + osz],
                        _canvas_view(yc[:], cb, 1, 1),
                        start=False, stop=False)
                    nc.tensor.matmul(
                        ps[:], m1T[:, o0:o0 + osz],
                        v_sb[:, cb * NCHUNK:(cb + 1) * NCHUNK],
                        start=False, stop=True)
                    osb = ostg.tile([osz, NCHUNK], F32, tag=f"os{mt}")
                    if mt == 0:
                        nc.scalar.copy(out=osb[:], in_=ps[:])
                    else:
                        nc.vector.tensor_copy(osb[:], ps[:])
                    oeng = nc.sync if cb % 2 == 0 else nc.scalar
                    oeng.dma_start(
                        out=out_d[o0:o0 + osz,
                                  cb * NCHUNK:(cb + 1) * NCHUNK],
                        in_=osb[:])


_NC_CACHE = None


def kernel(x, y, w_pos, w_qv, w_qv_dw, w_kv, w_kv_dw, w_proj, w_fuse, b_fuse,
           temperature):
    global _NC_CACHE, LAST_RESULTS
    x = _f32(np.asarray(x))
    y = _f32(np.asarray(y))
    wts = _prep_weights(
        np.asarray(w_pos, np.float32), np.asarray(w_qv, np.float32),
        np.asarray(w_qv_dw, np.float32), np.asarray(w_kv, np.float32),
        np.asarray(w_kv_dw, np.float32), np.asarray(w_proj, np.float32),
        np.asarray(w_fuse, np.float32), np.asarray(b_fuse, np.float32),
        np.asarray(temperature, np.float32))

    if _NC_CACHE is None:
        _NC_CACHE = _build_nc()
    nc = _NC_CACHE

    in_maps = []
    for core in range(B):
        m = {"x": np.ascontiguousarray(x[core].reshape(C, N)),
             "y": np.ascontiguousarray(y[core].reshape(C, N))}
        m.update(wts)
        in_maps.append(m)

    res = run_bass_kernel_spmd(nc, in_maps, core_ids=list(range(B)),
                               trace=TRACE)
    LAST_RESULTS = res
    out = np.stack([np.asarray(r["out"]) for r in res.results])
    return out.reshape(B, 2 * C, H, W).astype(np.float32)


if __name__ == "__main__":
    xs = np.random.randn(B, C, H, W).astype(np.float32)
    ys = np.random.randn(B, C, H, W).astype(np.float32)
    print("built nc ok" if _build_nc() else "")

